# revision 1
# baseline (speedup 1.0000x reference)
"""Hausdorff distance kernel for Trainium2 (8 NeuronCores, SPMD data-parallel over batch).

Problem: adv [16, 4096, 3] f32, ori [16, 4096, 3] f32 ->
  scalar mean over batch of max(max_n min_m D[n,m], max_m min_n D[n,m]),
  D = squared pairwise distances.

Strategy per core (2 batches):
  Phase 1 (fp16, fast): distance matrix via a K=13 matmul using the
  homogeneous-coordinate trick with fp16 hi/lo-split coordinates, so PSUM f32
  distances are accurate to ~1e-7 relative.  4 concurrent matmuls via
  tile_position row-group packing.  ACT copies PSUM f32 -> SBUF fp16;
  DVE does row-mins (tensor_scalar + accum_out, 4x mode) and running
  column-mins (tensor_tensor min, 2x mode).  fp16 storage rounding (~5e-4)
  only affects *candidate selection*, not the final answer.
  Phase 2 (exact): per-partition-slot argmax candidates (128 rows / side) are
  gathered by index and their exact rowmins recomputed from PSUM f32,
  giving an exactly-f32-accurate final answer.
"""

import numpy as np
from contextlib import ExitStack

B, N, M, D3 = 16, 4096, 4096, 3
NCORES = 8
BPC = B // NCORES  # batches per core = 2
P = 128
NT = N // P  # 32 n-tiles
MSUB = 512  # matmul moving free dim (one PSUM bank of f32)
GROUP_M = 1024  # m-columns per ACT/DVE op (2 PSUM banks)
NG = M // GROUP_M  # 2 groups
NSUB = GROUP_M // MSUB  # 4 matmuls (row-group packed) per group
KP = 13  # packed contraction rows
BIGVAL = 60000.0  # +inf stand-in for fp16 min-accumulators

_BUILD_CACHE = {}


def _split16(v):
    hi = v.astype(np.float16)
    lo = (v.astype(np.float32) - hi.astype(np.float32)).astype(np.float16)
    return hi, lo


def _pack_batch(x, y):
    """x=adv[b] [N,3] f32, y=ori[b] [M,3] f32 -> packed device arrays."""
    f16 = np.float16
    xh, xl = _split16(x)
    yh, yl = _split16(y)
    x2 = (x * x).sum(-1, dtype=np.float32)
    y2 = (y * y).sum(-1, dtype=np.float32)
    x2h, x2l = _split16(x2)
    y2h, y2l = _split16(y2)
    one = np.ones_like(x2h)

    # lhs packing (for "query" points q): rows pair with rhs packing of points p
    # sum_k lhs[k]*rhs[k] = |q|^2 + |p|^2 - 2(qh.ph + qh.pl + ql.ph)
    def lhs_pack(h, l, n2h, n2l):
        return np.stack([
            h[:, 0], h[:, 1], h[:, 2],
            h[:, 0], h[:, 1], h[:, 2],
            l[:, 0], l[:, 1], l[:, 2],
            n2h, n2l, one, one,
        ]).astype(f16)  # [13, N]

    def rhs_pack(h, l, n2h, n2l):
        m2 = np.float16(-2.0)
        return np.stack([
            m2 * h[:, 0], m2 * h[:, 1], m2 * h[:, 2],
            m2 * l[:, 0], m2 * l[:, 1], m2 * l[:, 2],
            m2 * h[:, 0], m2 * h[:, 1], m2 * h[:, 2],
            one, one, n2h, n2l,
        ]).astype(f16)  # [13, N]

    advL13 = lhs_pack(xh, xl, x2h, x2l)
    oriR13 = rhs_pack(yh, yl, y2h, y2l)
    oriL13 = lhs_pack(yh, yl, y2h, y2l)
    advR13 = rhs_pack(xh, xl, x2h, x2l)

    # 4x replication at partition offsets 0/32/64/96 for row-group packing
    advL = np.zeros([P, N], f16)
    oriR = np.zeros([P, M], f16)
    for r in range(4):
        advL[32 * r:32 * r + KP] = advL13
        oriR[32 * r:32 * r + KP] = oriR13
    advR = np.zeros([16, N], f16)
    advR[:KP] = advR13
    # gather sources for refinement (rows = lhs packing columns), f32 holding
    # exactly-fp16 values so the roundtrip through f32 is lossless
    advA = np.zeros([N, 16], np.float32)
    advA[:, :KP] = advL13.T.astype(np.float32)
    oriA = np.zeros([M, 16], np.float32)
    oriA[:, :KP] = oriL13.T.astype(np.float32)
    return advL, oriR, advR, advA, oriA


def _build_nc():
    import concourse.bass as bass
    import concourse.mybir as mybir
    import concourse.tile as tile
    from concourse.masks import make_identity

    dt = mybir.dt
    Alu = mybir.AluOpType
    Ax = mybir.AxisListType

    nc = bass.Bass("TRN2")

    advL_d = nc.dram_tensor("advL", [BPC, P, N], dt.float16, kind="ExternalInput")
    oriR_d = nc.dram_tensor("oriR", [BPC, P, M], dt.float16, kind="ExternalInput")
    advR_d = nc.dram_tensor("advR", [BPC, 16, N], dt.float16, kind="ExternalInput")
    advA_d = [nc.dram_tensor(f"advA{b}", [N, 16], dt.float32, kind="ExternalInput")
              for b in range(BPC)]
    oriA_d = [nc.dram_tensor(f"oriA{b}", [M, 16], dt.float32, kind="ExternalInput")
              for b in range(BPC)]
    iota_d = nc.dram_tensor("iotaP", [P, 1], dt.float32, kind="ExternalInput")
    loss_d = nc.dram_tensor("loss", [1, BPC], dt.float32, kind="ExternalOutput")

    with tile.TileContext(nc) as tc, ExitStack() as ctx:
        const = ctx.enter_context(tc.tile_pool(name="const", bufs=1))
        fresh_p = ctx.enter_context(tc.tile_pool(name="fresh", bufs=6))
        scr_p = ctx.enter_context(tc.tile_pool(name="scr", bufs=4))
        scrf_p = ctx.enter_context(tc.tile_pool(name="scrf", bufs=2))
        psum_p = ctx.enter_context(tc.tile_pool(name="psum", bufs=2, space="PSUM"))
        small_p = ctx.enter_context(tc.tile_pool(name="small", bufs=2))

        # ---- persistent inputs / constants ----
        advL_sb = [const.tile([P, N], dt.float16, tag=f"advL{b}", name=f"advL{b}") for b in range(BPC)]
        oriR_sb = [const.tile([P, M], dt.float16, tag=f"oriR{b}", name=f"oriR{b}") for b in range(BPC)]
        advR_sb = [const.tile([16, N], dt.float16, tag=f"advR{b}", name=f"advR{b}") for b in range(BPC)]
        for b in range(BPC):
            nc.sync.dma_start(advL_sb[b][:], advL_d[b, :, :])
            nc.sync.dma_start(oriR_sb[b][:], oriR_d[b, :, :])
            nc.sync.dma_start(advR_sb[b][:], advR_d[b, :, :])
        iota_sb = const.tile([P, 1], dt.float32, tag="iota")
        nc.sync.dma_start(iota_sb[:], iota_d[:])
        ident16 = const.tile([P, P], dt.float16, tag="id16")
        make_identity(nc, ident16[:])
        identF = const.tile([P, P], dt.float32, tag="idF")
        make_identity(nc, identF[:])

        colacc = const.tile([P, M], dt.float16, tag="colacc")
        rowbufF = const.tile([P, NT * NG], dt.float32, tag="rowbufF")
        loss_sb = const.tile([1, BPC], dt.float32, tag="loss_sb")

        def refine(b, valsT, lhs_gather_dram, rhs_sb, side):
            """valsT [P, 32] f32: value for point index c*128+p at [p, c].
            Returns [1,1] f32 tile with exact max-of-min for candidate rows."""
            v8 = small_p.tile([P, 8], dt.float32, tag=f"v8")
            nc.vector.max(v8[:], valsT[:])
            i8 = small_p.tile([P, 8], dt.uint32, tag=f"i8")
            nc.vector.max_index(i8[:], v8[:], valsT[:])
            tf = small_p.tile([P, 1], dt.float32, tag=f"tf")
            nc.vector.tensor_copy(out=tf[:], in_=i8[:, 0:1])
            nf = small_p.tile([P, 1], dt.float32, tag=f"nf")
            nc.vector.tensor_scalar(
                out=nf[:], in0=tf[:], scalar1=128.0, scalar2=iota_sb[:],
                op0=Alu.mult, op1=Alu.add)
            idx = small_p.tile([P, 1], dt.uint32, tag=f"idx")
            nc.vector.tensor_copy(out=idx[:], in_=nf[:])
            cand = small_p.tile([P, 16], dt.float32, tag=f"cand")
            nc.gpsimd.indirect_dma_start(
                out=cand[:], out_offset=None,
                in_=lhs_gather_dram[:],
                in_offset=bass.IndirectOffsetOnAxis(ap=idx[:, 0:1], axis=0))
            cand2 = small_p.tile([P, 16], dt.float32, tag=f"cand2")
            nc.vector.tensor_copy(out=cand2[:], in_=cand[:])
            candT_ps = psum_p.tile([16, P], dt.float32, tag="ct", name="candT", bufs=2)
            nc.tensor.transpose(candT_ps[:], cand2[:], identF[:])
            candL = small_p.tile([16, P], dt.float16, tag=f"candL")
            nc.scalar.copy(candL[:], candT_ps[:])
            cminb = small_p.tile([P, NG], dt.float32, tag=f"cminb")
            for g in range(NG):
                ptR = psum_p.tile([P, GROUP_M], dt.float32, tag="pt")
                for j in range(NSUB):
                    ms = g * GROUP_M + j * MSUB
                    nc.tensor.matmul(
                        ptR[:, j * MSUB:(j + 1) * MSUB],
                        lhsT=candL[0:KP, :],
                        rhs=rhs_sb[0:KP, ms:ms + MSUB],
                        start=True, stop=True, tile_position=(0, 0))
                scrF = scrf_p.tile([P, GROUP_M], dt.float32, tag="scrF")
                nc.vector.tensor_scalar(
                    out=scrF[:], in0=ptR[:], scalar1=0.0, scalar2=None,
                    op0=Alu.bypass, op1=Alu.min,
                    accum_out=cminb[:, g:g + 1])
            cmin = small_p.tile([P, 1], dt.float32, tag=f"cmin")
            nc.vector.tensor_reduce(cmin[:], cminb[:], axis=Ax.X, op=Alu.min)
            cmT = psum_p.tile([1, P], dt.float32, tag="ct", name="cmT", bufs=2)
            nc.tensor.transpose(cmT[:], cmin[:], identF[:])
            a_side = small_p.tile([1, 1], dt.float32, tag=f"aside_{side}")
            nc.vector.tensor_reduce(a_side[:], cmT[:], axis=Ax.X, op=Alu.max)
            return a_side

        for b in range(BPC):
            nc.vector.memset(colacc[:], BIGVAL)
            for t in range(NT):
                for g in range(NG):
                    pt = psum_p.tile([P, GROUP_M], dt.float32, tag="pt")
                    for i in range(NSUB):
                        ms = (g * NSUB + i) * MSUB
                        nc.tensor.matmul(
                            pt[:, i * MSUB:(i + 1) * MSUB],
                            lhsT=advL_sb[b][32 * i:32 * i + KP, t * P:(t + 1) * P],
                            rhs=oriR_sb[b][32 * i:32 * i + KP, ms:ms + MSUB],
                            start=True, stop=True, tile_position=(32 * i, 0))
                    fresh = fresh_p.tile([P, GROUP_M], dt.float16, tag="fresh")
                    nc.scalar.copy(fresh[:], pt[:])
                    scr = scr_p.tile([P, GROUP_M], dt.float16, tag="scr")
                    nc.vector.tensor_scalar(
                        out=scr[:], in0=fresh[:], scalar1=0.0, scalar2=None,
                        op0=Alu.bypass, op1=Alu.min,
                        accum_out=rowbufF[:, t * NG + g:t * NG + g + 1])
                    csl = colacc[:, g * GROUP_M:(g + 1) * GROUP_M]
                    nc.vector.tensor_tensor(out=csl, in0=fresh[:], in1=csl, op=Alu.min)

            # ---- row-side finals: rowfinal[p, t] = min over g ----
            rowfinal = small_p.tile([P, NT], dt.float32, tag="rowfinal")
            nc.vector.tensor_reduce(
                rowfinal[:], rowbufF[:, :].rearrange("p (t g) -> p t g", g=NG),
                axis=Ax.X, op=Alu.min)
            a_row = refine(b, rowfinal, advA_d[b], oriR_sb[b][:], "row")

            # ---- col-side finals: transpose colacc chunks, reduce over n ----
            colminT = small_p.tile([P, NT], dt.float32, tag="colminT")
            for cq in range(M // (4 * P)):  # 8 quads of 4 chunks
                cps = psum_p.tile([P, 4 * P], dt.float16, tag="cps", name="cps", bufs=2)
                for cc in range(4):
                    c = cq * 4 + cc
                    nc.tensor.transpose(
                        cps[:, cc * P:(cc + 1) * P],
                        colacc[:, c * P:(c + 1) * P], ident16[:])
                nc.vector.tensor_reduce(
                    colminT[:, cq * 4:(cq + 1) * 4],
                    cps[:, :].rearrange("p (c q) -> p c q", q=P),
                    axis=Ax.X, op=Alu.min)
            a_col = refine(b, colminT, oriA_d[b], advR_sb[b][:], "col")

            nc.vector.tensor_tensor(
                out=loss_sb[:, b:b + 1], in0=a_row[:], in1=a_col[:], op=Alu.max)

        nc.sync.dma_start(loss_d[:], loss_sb[:])

    # Strip same-engine self-waits from instructions that carry >=2 waits:
    # engines execute their own queue serially and in order, so a wait on the
    # engine's own completion semaphore is implied by program order.  This
    # keeps instructions within the ISA structs' limited sync-wait slots.
    _eng_sem = {
        mybir.EngineType.PE: "PE_",
        mybir.EngineType.Activation: "Activation_",
        mybir.EngineType.DVE: "DVE_",
        mybir.EngineType.Pool: "Pool_",
        mybir.EngineType.SP: "SP_",
    }
    for f in nc.m.functions:
        for bb in f.blocks:
            for i in bb.instructions:
                si = i.sync_info
                pref = _eng_sem.get(i.engine)
                if si is None or pref is None:
                    continue
                waits = list(si.on_wait)
                if len(waits) >= 2:
                    keep = [w for w in waits if not (w.ant_name or "").startswith(pref)]
                    if len(keep) < len(waits) and len(keep) >= 1:
                        i.sync_info = mybir.SyncInfo(
                            on_wait=keep, on_update=list(si.on_update))
    # The per-engine ISA structs hold a single sync-wait slot.  Sequencer-only
    # NOPs can wait on arbitrarily many semaphores, so spill all but one wait
    # of any multi-wait compute instruction onto a NOP inserted just before it
    # on the same engine queue.
    _compute_engines = {mybir.EngineType.PE, mybir.EngineType.Activation,
                        mybir.EngineType.DVE, mybir.EngineType.Pool}
    for f in nc.m.functions:
        for bb in f.blocks:
            newinsts = []
            for i in bb.instructions:
                si = i.sync_info
                if (si is not None
                        and i.opcode not in ("NoOp",)
                        and len(si.on_wait) >= 2):
                    waits = list(si.on_wait)
                    for wi, w in enumerate(waits[:-1]):
                        nop = mybir.InstNoOp(
                            name=f"I-waitnop{wi}-{i.name}", ins=[], outs=[])
                        nop.engine = i.engine
                        nop.sync_info = mybir.SyncInfo(on_wait=[w], on_update=[])
                        newinsts.append(nop)
                    i.sync_info = mybir.SyncInfo(
                        on_wait=waits[-1:], on_update=list(si.on_update))
                newinsts.append(i)
            bb.set_instructions_from_list(newinsts) if hasattr(bb, "set_instructions_from_list") else None
            if not hasattr(bb, "set_instructions_from_list"):
                bb.instructions = newinsts
    nc.finalize()
    return nc


def _get_nc():
    if "nc" not in _BUILD_CACHE:
        _BUILD_CACHE["nc"] = _build_nc()
    return _BUILD_CACHE["nc"]


def _make_in_maps(adv, ori):
    adv = np.ascontiguousarray(adv, dtype=np.float32)
    ori = np.ascontiguousarray(ori, dtype=np.float32)
    iota = np.arange(P, dtype=np.float32).reshape(P, 1)
    in_maps = []
    for c in range(NCORES):
        advL = np.zeros([BPC, P, N], np.float16)
        oriR = np.zeros([BPC, P, M], np.float16)
        advR = np.zeros([BPC, 16, N], np.float16)
        m = {}
        for b in range(BPC):
            gb = c * BPC + b
            aL, oR, aR, aA, oA = _pack_batch(adv[gb], ori[gb])
            advL[b], oriR[b], advR[b] = aL, oR, aR
            m[f"advA{b}"] = aA
            m[f"oriA{b}"] = oA
        m["advL"] = advL
        m["oriR"] = oriR
        m["advR"] = advR
        m["iotaP"] = iota
        in_maps.append(m)
    return in_maps


def kernel(adv, ori):
    from concourse.bass_utils import run_bass_kernel_spmd

    nc = _get_nc()
    in_maps = _make_in_maps(adv, ori)
    res = run_bass_kernel_spmd(nc, in_maps, core_ids=list(range(NCORES)))
    losses = np.concatenate([r["loss"].reshape(-1) for r in res.results])
    return np.float32(np.mean(losses.astype(np.float32)))



# revision 28
# speedup vs baseline: 5.0323x; 5.0323x over previous
"""Hausdorff distance kernel for Trainium2 (8 NeuronCores, SPMD over batch).

Problem: adv [16, 4096, 3] f32, ori [16, 4096, 3] f32 ->
  scalar mean over batch of max(max_n min_m D[n,m], max_m min_n D[n,m]),
  D = squared pairwise distances.  Harness gate: rel_err < 2e-2.

Strategy per core (2 batches, 4 "units" = (batch, side); each side is a pure
row-problem max_n min_m over its own distance matrix):
  Packing (K=6, plain f16): lhs rows [x0,x1,x2, 1, x2h, x2l], rhs rows
  [-2y0,-2y1,-2y2, y2h, 1, 1] so PSUM f32 holds the full squared distance
  (x2 folded into the matmul via an f16 hi/lo split; value error ~1e-3).
  Phase 1 (sampled): distances vs every-16th rhs point -> per-row upper
  bounds of the row-min, accumulated per n-tile into rowbuf [128, 32].
  Selection: per-partition-slot top-2 via DVE max8/max_index (a tiny
  per-tile epsilon makes values unique so indices are distinct), giving
  256 candidate rows; candidate packing rows are gathered by index
  (gpsimd indirect DMA) and transposed via the DMA xbar.
  Phase 2 (exact): candidates' full row-mins over all 4096 columns; the
  raw per-candidate segment mins [128, 64] are shipped out and the host
  does the final min/max reduction and the batch mean.
  Drains (the bottleneck) are split between ACT (PSUM f32 -> SBUF f16
  copy, then DVE 4x-mode min-accum) and DVE-direct (PSUM f32 1x min-accum).
"""

import numpy as np
from contextlib import ExitStack

B, N, M, D3 = 16, 4096, 4096, 3
NCORES = 8
BPC = B // NCORES        # batches per core = 2
NU = 2 * BPC             # units per core = (batch, side) = 4
P = 128
NT = N // P              # 32 n-tiles
S = 16                   # column sampling stride for phase 1
MS = M // S              # 256 sampled columns
KP = 6                   # packed contraction rows
TOPK = 2                 # candidates per partition slot
GW = 128                 # gather width (f16 cols); xbar transpose needs free%128==0

# drain-path assignment (tuned against the cost model):
# phase 1: most round-pairs drain via ACT copy + DVE 4x accums; a few go
# DVE-direct to shorten the serial ACT wall.  (u, pair) -> via ACT?
P1_DVE_PAIRS = set()
# route these (r, i) phase-1 accum segments to gpsimd to free DVE capacity
P1_POOL_SEGS = set()
# per-unit (g,chunk) phase-2 drain: 'T' = ACT copy + DVE accums,
# 'H' = hybrid (ACT copies banks 0-1, DVE direct-drains banks 2-3)
P2_KIND = [
    ("3", "3", "3", "3"),
    ("3", "3", "3", "3"),
    ("3", "3", "3", "3"),
    ("3", "3", "3", "T"),
]
# emission order: all phase-1 units (with selections) first, then phase-2
SCHEDULE = ["p1_0", "p1_1", "p1_2", "p1_3", "p2_0", "p2_1", "p2_2", "p2_3"]

_BUILD_CACHE = {}


def _pack_side(x, y):
    """x: lhs points [N,3] f32, y: rhs points [M,3] f32.
    Returns (lhs [128,N] f16, rhs_full [128,M] f16, rhs_s [128,MS] f16,
             gather [N,GW] f16)."""
    f16 = np.float16
    xh = x.astype(f16)                       # [N,3]
    yh = y.astype(f16)
    x2 = (x * x).sum(-1, dtype=np.float32)   # [N]
    x2h = x2.astype(f16)
    x2l = (x2 - x2h.astype(np.float32)).astype(f16)
    y2h = (y * y).sum(-1, dtype=np.float32).astype(f16)
    one = np.ones(N, f16)

    lhs6 = np.stack([xh[:, 0], xh[:, 1], xh[:, 2], one, x2h, x2l])      # [6,N]
    m2 = f16(-2.0)
    rhs6 = np.stack([m2 * yh[:, 0], m2 * yh[:, 1], m2 * yh[:, 2],
                     y2h, np.ones(M, f16), np.ones(M, f16)])            # [6,M]

    lhs = np.zeros([P, N], f16)
    rhsf = np.zeros([P, M], f16)
    rhss = np.zeros([P, MS], f16)
    for r in range(4):
        lhs[32 * r:32 * r + KP] = lhs6
        rhsf[32 * r:32 * r + KP] = rhs6
        rhss[32 * r:32 * r + KP] = rhs6[:, ::S]
    gat = np.zeros([N, GW], f16)
    for r in range(4):
        gat[:, 32 * r:32 * r + KP] = lhs6.T
    return lhs, rhsf, rhss, gat


def _build_nc():
    import concourse.bass as bass
    import concourse.mybir as mybir
    import concourse.tile as tile

    dt = mybir.dt
    Alu = mybir.AluOpType

    nc = bass.Bass("TRN2")

    lhs_d = nc.dram_tensor("lhs", [NU, P, N], dt.float16, kind="ExternalInput")
    rhsf_d = nc.dram_tensor("rhsf", [NU, P, M], dt.float16, kind="ExternalInput")
    rhss_d = nc.dram_tensor("rhss", [NU, P, MS], dt.float16, kind="ExternalInput")
    gat_d = [nc.dram_tensor(f"gat{u}", [N, GW], dt.float16, kind="ExternalInput")
             for u in range(NU)]
    iota_d = nc.dram_tensor("iotaP", [P, 1], dt.float32, kind="ExternalInput")
    eps_d = nc.dram_tensor("epsT", [P, NT], dt.float32, kind="ExternalInput")
    loss_d = nc.dram_tensor("loss", [P, NU * 16], dt.float32, kind="ExternalOutput")

    with tile.TileContext(nc) as tc, ExitStack() as ctx:
        const = ctx.enter_context(tc.tile_pool(name="const", bufs=1))
        fresh_p = ctx.enter_context(tc.tile_pool(name="fresh", bufs=4))
        small_p = ctx.enter_context(tc.tile_pool(name="small", bufs=2))
        psum_p = ctx.enter_context(tc.tile_pool(name="psum", bufs=2, space="PSUM"))

        lhs_sb = [const.tile([P, N], dt.float16, tag=f"lhs{u}") for u in range(NU)]
        rhss_sb = [const.tile([P, MS], dt.float16, tag=f"rhss{u}") for u in range(NU)]
        rhsf_sb = [const.tile([P, M], dt.float16, tag=f"rhsf{u}") for u in range(NU)]
        iota_sb = const.tile([P, 1], dt.float32, tag="iota")
        eps_sb = const.tile([P, NT], dt.float32, tag="eps")
        rowbuf = [const.tile([P, NT], dt.float32, tag=f"rowbuf{u}") for u in range(NU)]
        excol = [const.tile([P, 2 * 2 * 4], dt.float32, tag=f"excol{u}") for u in range(NU)]
        candL = [[const.tile([P, P], dt.float16, tag=f"candL{u}_{g}")
                  for g in range(TOPK)] for u in range(NU)]
        exmin = const.tile([P, NU * TOPK], dt.float32, tag="exmin")
        fmax16 = const.tile([P, 32], dt.float16, tag="fmax16")
        fred = const.tile([32, 1], dt.float32, tag="fred")

        # input DMAs: lhs0 first half + small tensors on SP (fast start);
        # bulk tensors via gpsimd SWDGE so SP stays free for the transposes
        # and the transfers ride the DMA engines in the background.
        QN = N // 4
        nc.scalar.dma_start(rhss_sb[0][:], rhss_d[0, :, :])
        nc.sync.dma_start(lhs_sb[0][:, 0:QN // 2], lhs_d[0, :, 0:QN // 2])
        nc.sync.dma_start(lhs_sb[0][:, QN // 2:QN], lhs_d[0, :, QN // 2:QN])
        nc.sync.dma_start(lhs_sb[0][:, QN:2 * QN], lhs_d[0, :, QN:2 * QN])
        nc.sync.dma_start(iota_sb[:], iota_d[:])
        nc.sync.dma_start(eps_sb[:], eps_d[:])
        for u in range(1, NU):
            nc.sync.dma_start(rhss_sb[u][:], rhss_d[u, :, :])
        for u in range(NU):
            nc.sync.dma_start(rhsf_sb[u][:], rhsf_d[u, :, :])
        nc.gpsimd.dma_start(lhs_sb[0][:, 2 * QN:N], lhs_d[0, :, 2 * QN:N])
        for u in range(1, NU):
            nc.gpsimd.dma_start(lhs_sb[u][:], lhs_d[u, :, :])
        nc.vector.memset(scr11[0:1, 0:1], 0.0)
        nc.scalar.copy(scr11[0:1, 1:2], scr11[0:1, 0:1])

        # ---- phase 1 + selection, unit-major so selections overlap ----
        def phase1_and_select(u):
            # f16-accums of an ACT pair are emitted one pair late so that a
            # DVE-direct pair's PSUM accums don't queue behind them (they
            # would stall PSUM recycling waiting on the previous ACT copy).
            pending = []
            for pairidx in range(NT // 8):
                pt = psum_p.tile([P, 2048], dt.float32, tag="pt", name="pt")
                for r in range(2):
                    for i in range(4):
                        t = pairidx * 8 + r * 4 + i
                        nc.tensor.matmul(
                            pt[:, i * 512 + r * 256:i * 512 + r * 256 + 256],
                            lhsT=lhs_sb[u][32 * i:32 * i + KP, t * P:(t + 1) * P],
                            rhs=rhss_sb[u][32 * i:32 * i + KP, :],
                            start=True, stop=True, tile_position=(32 * i, 0))
                via_act = (u, pairidx) not in P1_DVE_PAIRS
                if via_act:
                    fr = fresh_p.tile([P, 2048], dt.float16, tag="fresh", name="fresh")
                    nc.scalar.copy(fr[:], pt[:])
                    def accums(fr=fr, pairidx=pairidx, u=u):
                        for i in range(4):
                            for r in range(2):
                                t = pairidx * 8 + r * 4 + i
                                sl = fr[:, i * 512 + r * 256:i * 512 + r * 256 + 256]
                                eng = nc.gpsimd if (r, i) in P1_POOL_SEGS else nc.vector
                                eng.tensor_scalar(
                                    out=sl, in0=sl, scalar1=0.0, scalar2=None,
                                    op0=Alu.bypass, op1=Alu.min,
                                    accum_out=rowbuf[u][:, t:t + 1])
                    pending.append(accums)
                else:
                    for i in range(4):
                        for r in range(2):
                            t = pairidx * 8 + r * 4 + i
                            sl = pt[:, i * 512 + r * 256:i * 512 + r * 256 + 256]
                            fr2 = fresh_p.tile([P, 256], dt.float16, tag="dump", name="dump")
                            nc.vector.tensor_scalar(
                                out=fr2[:], in0=sl, scalar1=0.0, scalar2=None,
                                op0=Alu.bypass, op1=Alu.min,
                                accum_out=rowbuf[u][:, t:t + 1])
                while len(pending) > 1:
                    pending.pop(0)()
            for fn in pending:
                fn()

            # selection: per-slot top-2 by upper bound
            nc.vector.tensor_tensor(out=rowbuf[u][:], in0=rowbuf[u][:],
                                    in1=eps_sb[:], op=Alu.add)
            v8 = small_p.tile([P, 8], dt.float32, tag=f"v8_{u}", name=f"v8_{u}")
            nc.vector.max(v8[:], rowbuf[u][:])
            i8 = small_p.tile([P, 8], dt.uint32, tag=f"i8_{u}", name=f"i8_{u}")
            nc.vector.max_index(i8[:], v8[:], rowbuf[u][:])
            tf = small_p.tile([P, TOPK], dt.float32, tag=f"tf_{u}", name=f"tf_{u}")
            nc.vector.tensor_copy(out=tf[:], in_=i8[:, 0:TOPK])
            nf = small_p.tile([P, TOPK], dt.float32, tag=f"nf_{u}", name=f"nf_{u}")
            nc.vector.tensor_scalar(
                out=nf[:], in0=tf[:], scalar1=float(P), scalar2=iota_sb[:],
                op0=Alu.mult, op1=Alu.add)
            idx = small_p.tile([P, TOPK], dt.uint32, tag=f"idx_{u}", name=f"idx_{u}")
            nc.vector.tensor_copy(out=idx[:], in_=nf[:])
            for g in range(TOPK):
                cand = small_p.tile([P, GW], dt.float16, tag=f"cand_{u}_{g}", name=f"cand_{u}_{g}")
                nc.gpsimd.indirect_dma_start(
                    out=cand[:], out_offset=None,
                    in_=gat_d[u][:],
                    in_offset=bass.IndirectOffsetOnAxis(ap=idx[:, g:g + 1], axis=0))
                nc.sync.dma_start_transpose(candL[u][g][:], cand[:])

        def phase2(u):
            for g in range(TOPK):
                for chunk in range(2):
                    pt = psum_p.tile([P, 2048], dt.float32, tag="pt", name="pt")
                    for i in range(4):
                        mc = chunk * 2048 + i * 512
                        nc.tensor.matmul(
                            pt[:, i * 512:(i + 1) * 512],
                            lhsT=candL[u][g][32 * i:32 * i + KP, :],
                            rhs=rhsf_sb[u][32 * i:32 * i + KP, mc:mc + 512],
                            start=True, stop=True, tile_position=(32 * i, 0))
                    base = u * 16 + g * 8 + chunk * 4
                    kind = P2_KIND[u][g * 2 + chunk]
                    if kind == "F":
                        for i in range(4):
                            fr2 = fresh_p.tile([P, 512], dt.float16, tag="dump2", name="dump2")
                            nc.vector.tensor_scalar(
                                out=fr2[:], in0=pt[:, i * 512:(i + 1) * 512],
                                scalar1=0.0, scalar2=None,
                                op0=Alu.bypass, op1=Alu.min,
                                accum_out=excol[:, base + i:base + i + 1])
                    elif kind == "T":
                        fr = fresh_p.tile([P, 2048], dt.float16, tag="fresh", name="fresh")
                        nc.scalar.copy(fr[:], pt[:])
                        for i in range(4):
                            sl = fr[:, i * 512:(i + 1) * 512]
                            nc.vector.tensor_scalar(
                                out=sl, in0=sl, scalar1=0.0, scalar2=None,
                                op0=Alu.bypass, op1=Alu.min,
                                accum_out=excol[:, base + i:base + i + 1])
                    elif kind == "3":
                        # ACT copies banks 0-2, DVE direct-drains bank 3
                        fr = fresh_p.tile([P, 1536], dt.float16, tag="fresh3", name="fresh3")
                        nc.scalar.copy(fr[:], pt[:, 0:1536])
                        for i in (3,):
                            fr2 = fresh_p.tile([P, 512], dt.float16, tag="dump2", name="dump2")
                            nc.vector.tensor_scalar(
                                out=fr2[:], in0=pt[:, i * 512:(i + 1) * 512],
                                scalar1=0.0, scalar2=None,
                                op0=Alu.bypass, op1=Alu.min,
                                accum_out=excol[:, base + i:base + i + 1])
                        for i in (0, 1, 2):
                            sl = fr[:, i * 512:(i + 1) * 512]
                            nc.vector.tensor_scalar(
                                out=sl, in0=sl, scalar1=0.0, scalar2=None,
                                op0=Alu.bypass, op1=Alu.min,
                                accum_out=excol[:, base + i:base + i + 1])
                    else:
                        # hybrid: ACT copies banks 0-1 while DVE direct-drains
                        # banks 2-3 (different banks -> parallel access)
                        fr = fresh_p.tile([P, 1024], dt.float16, tag="freshh", name="freshh")
                        nc.scalar.copy(fr[:], pt[:, 0:1024])
                        for i in (2, 3):
                            fr2 = fresh_p.tile([P, 512], dt.float16, tag="dump2", name="dump2")
                            nc.vector.tensor_scalar(
                                out=fr2[:], in0=pt[:, i * 512:(i + 1) * 512],
                                scalar1=0.0, scalar2=None,
                                op0=Alu.bypass, op1=Alu.min,
                                accum_out=excol[:, base + i:base + i + 1])
                        for i in (0, 1):
                            sl = fr[:, i * 512:(i + 1) * 512]
                            nc.vector.tensor_scalar(
                                out=sl, in0=sl, scalar1=0.0, scalar2=None,
                                op0=Alu.bypass, op1=Alu.min,
                                accum_out=excol[:, base + i:base + i + 1])

        for step in SCHEDULE:
            kind_s, un = step.split("_")
            if kind_s == "p1":
                phase1_and_select(int(un))
            else:
                phase2(int(un))

        # ---- tail: ship raw per-candidate segment mins; host reduces ----
        nc.sync.dma_start(loss_d[:], excol[:])

    # Strip same-engine self-waits from instructions that carry >=2 waits:
    # engines execute their own queue serially and in order, so a wait on the
    # engine's own completion semaphore is implied by program order.  This
    # keeps instructions within the ISA structs' limited sync-wait slots.
    _eng_sem = {
        mybir.EngineType.PE: "PE_",
        mybir.EngineType.Activation: "Activation_",
        mybir.EngineType.DVE: "DVE_",
        mybir.EngineType.Pool: "Pool_",
        mybir.EngineType.SP: "SP_",
    }
    for f in nc.m.functions:
        for bb in f.blocks:
            for i in bb.instructions:
                si = i.sync_info
                pref = _eng_sem.get(i.engine)
                if si is None or pref is None:
                    continue
                waits = list(si.on_wait)
                if len(waits) >= 2:
                    keep = [w for w in waits if not (w.ant_name or "").startswith(pref)]
                    if len(keep) < len(waits) and len(keep) >= 1:
                        i.sync_info = mybir.SyncInfo(
                            on_wait=keep, on_update=list(si.on_update))
    # The per-engine ISA structs hold a single sync-wait slot.  Sequencer-only
    # NOPs can wait on arbitrarily many semaphores, so spill all but one wait
    # of any multi-wait compute instruction onto a NOP inserted just before it
    # on the same engine queue.
    for f in nc.m.functions:
        for bb in f.blocks:
            newinsts = []
            for i in bb.instructions:
                si = i.sync_info
                if (si is not None
                        and i.opcode not in ("NoOp",)
                        and len(si.on_wait) >= 2):
                    waits = list(si.on_wait)
                    for wi, w in enumerate(waits[:-1]):
                        nop = mybir.InstNoOp(
                            name=f"I-waitnop{wi}-{i.name}", ins=[], outs=[])
                        nop.engine = i.engine
                        nop.sync_info = mybir.SyncInfo(on_wait=[w], on_update=[])
                        newinsts.append(nop)
                    i.sync_info = mybir.SyncInfo(
                        on_wait=waits[-1:], on_update=list(si.on_update))
                newinsts.append(i)
            bb.instructions = newinsts
    nc.finalize()
    return nc


def _get_nc():
    if "nc" not in _BUILD_CACHE:
        _BUILD_CACHE["nc"] = _build_nc()
    return _BUILD_CACHE["nc"]


def _make_in_maps(adv, ori):
    adv = np.ascontiguousarray(adv, dtype=np.float32)
    ori = np.ascontiguousarray(ori, dtype=np.float32)
    iota = np.arange(P, dtype=np.float32).reshape(P, 1)
    eps = np.broadcast_to((np.arange(NT, dtype=np.float32) * 2e-6)[None, :],
                          (P, NT)).copy()
    in_maps = []
    for c in range(NCORES):
        lhs = np.zeros([NU, P, N], np.float16)
        rhsf = np.zeros([NU, P, M], np.float16)
        rhss = np.zeros([NU, P, MS], np.float16)
        m = {"iotaP": iota, "epsT": eps}
        for b in range(BPC):
            gb = c * BPC + b
            for side in range(2):
                u = b * 2 + side
                x, y = (adv[gb], ori[gb]) if side == 0 else (ori[gb], adv[gb])
                l, rf, rs, ga = _pack_side(x, y)
                lhs[u], rhsf[u], rhss[u] = l, rf, rs
                m[f"gat{u}"] = ga
        m["lhs"] = lhs
        m["rhsf"] = rhsf
        m["rhss"] = rhss
        in_maps.append(m)
    return in_maps


def kernel(adv, ori):
    from concourse.bass_utils import run_bass_kernel_spmd

    nc = _get_nc()
    in_maps = _make_in_maps(adv, ori)
    res = run_bass_kernel_spmd(nc, in_maps, core_ids=list(range(NCORES)))
    losses = []
    for r in res.results:
        # [128, NU*16] -> (slot, unit, group, chunk*bank): row-min = min over
        # the 8 segment mins; unit value = max over slots and groups
        v = r["loss"].reshape(P, NU, TOPK, 8).min(axis=3).max(axis=(0, 2))  # [NU]
        for b in range(BPC):
            losses.append(max(float(v[2 * b]), float(v[2 * b + 1])))
    return np.float32(np.mean(losses))
